# revision 16
# baseline (speedup 1.0000x reference)
"""Trainium2 Bass kernel for nn_BiLSTM_3410204033194.

The reference computes a 3-layer bidirectional LSTM over (T=1024, B=512,
IN=2) and applies the final FC to out[:, -1, :] — the LAST BATCH ELEMENT
only.  LSTM batch elements are independent, so the full output (T, 4)
depends only on batch index 511: we run the whole 3-layer bidirectional
recurrence for that single sequence on one core (all 8 cores run the same
SPMD program; core 0's output is used).

Instead of a step-by-step scan (latency-bound: ~1.5-2.5us per step x 3072
steps), each layer-direction is solved by PARALLEL-IN-TIME fixed-point
(Picard) iteration, which converges geometrically at the LSTM's
contraction rate (~0.28/sweep on this data; K sweeps give ~0.3^K error,
validated end-to-end in fp64/fp32 prototypes at <3e-6 for K=10):

    H^0 = 0
    repeat K times:
        A   = [H^{k-1} shifted by one step; X; 1] @ Waug     (PE, fp32r)
        S   = sigmoid(A)          all 4 gates; tanh(y)=2*sig(2y)-1 with
                                  the x2 folded into Waug's g columns
        U   = (S_g - .5) * S_i                                (DVE STT)
        C   = scan: c_t = S_f[t]*c_{t-1} + U[t]   (DVE tensor_tensor_scan,
                                                   ONE instr for all T)
        S_c = sigmoid(4*C)                                    (ACT)
        H^k = (S_c - .5) * S_o                                (DVE STT)

All state tensors carry h~ = h/2 and c~ = c/2 (the tanh-as-sigmoid
halves); the x2 is folded into every consumer's weights (W_hh, the next
layer's W_ih, and the FC).

The one-step shift is free: H rows of the GEMM's rhs tile R hold h~(t-1)
at column t (the H-update writes columns 1..T), while the X rows hold
x(t) at column t.  The bias rides in Waug against an all-ones row of R.

Partition layout (hardware rule: operand partition starts in {0,32,64,96},
tensor ops' inputs share a start partition):
  psum quads   dir f: f@0 i@32 o@64 g@96      dir b: i@0 f@32 g@64 o@96
  sig1: ps[0:52]->SA_d[64:116]   f:(f@64,i@96)  b:(i@64,f@96)
  sig2: ps[64:116]->SB_d[64:116] f:(o@64,g@96)  b:(g@64,o@96)
  U_f=(SB_f[96:]-.5)*SA_f[96:]->U[64:84]      U_b=(SB_b[64:84]-.5)*SA_b[64:84]->U[96:116]
  TTS_f: (SA_f[64:84], U[64:84])->CT[0:20]    TTS_b: (SA_b[96:116], U[96:116])->CT[32:52]
  sigc: CT[0:52] -> SC[64:116]
  H_f=(SC[64:84]-.5)*SB_f[64:84]->R_f[0:20,1:] H_b=(SC[96:116]-.5)*SB_b[96:116]->R_b[0:20,1:]
"""
import os
import sys

sys.path.insert(0, "/opt/trn_rl_repo")

import numpy as np
from contextlib import ExitStack

import concourse.bass as bass
import concourse.tile as tile
from concourse import mybir
from concourse.bass_utils import run_bass_kernel_spmd

F32 = mybir.dt.float32
F32R = mybir.dt.float32r
AF = mybir.ActivationFunctionType
ALU = mybir.AluOpType

H = 20
NCORES = 8
K_ITERS = 5
# quad (x32) of each pytorch gate, same for both directions
QUAD = {"f": 0, "i": 1, "o": 2, "g": 3}
GATE_ROWS = {"i": 0, "f": 1, "g": 2, "o": 3}  # row blocks in pytorch weights


# ---------------------------------------------------------------- host prep
def _make_lhsT(w_hh, w_ih, b, h_fold_x):
    """Build the augmented stationary (20+D+1, 116).

    rows 0..19   : W_hh^T * 2           (consumes h~ = h/2)
    rows 20..+D  : W_ih^T * h_fold_x    (2 if the layer input is h~ tiles)
    row  20+D    : bias
    columns      : gate quads per `quad`, g columns additionally * 2
                   (tanh(y) = 2*sigmoid(2y) - 1).
    """
    d = w_ih.shape[1]
    out = np.zeros((97, 116), np.float32)
    for gate, gi in GATE_ROWS.items():
        rows = slice(H * gi, H * (gi + 1))
        c0 = 32 * QUAD[gate]
        out[0:H, c0:c0 + H] = w_hh[rows].T * 2.0
        if d == 2:
            out[32:34, c0:c0 + H] = w_ih[rows].T * h_fold_x
        else:
            out[32:52, c0:c0 + H] = w_ih[rows, 0:H].T * h_fold_x
            out[64:84, c0:c0 + H] = w_ih[rows, H:2 * H].T * h_fold_x
        out[96, c0:c0 + H] = b[rows]
    return out


def prep_inputs(x, w_ih0, w_hh0, b0, w_ih12, w_hh12, b12, fc_w, fc_b, t_len):
    arrs = {}
    xs = np.asarray(x[:t_len, -1, :], np.float32)     # (T, 2)
    arrs["x_f"] = np.ascontiguousarray(xs.T)          # (2, T)
    arrs["x_r"] = np.ascontiguousarray(xs[::-1].T)    # (2, T) reversed time
    arrs["ones1"] = np.ones((1, t_len), np.float32)
    for l in range(3):
        for d in range(2):
            if l == 0:
                wih, whh, bb = w_ih0[d], w_hh0[d], b0[d]
                fold = 1.0
            else:
                wih, whh, bb = w_ih12[l - 1, d], w_hh12[l - 1, d], b12[l - 1, d]
                fold = 2.0
            arrs[f"w_{l}_{d}"] = _make_lhsT(
                np.asarray(whh, np.float32), np.asarray(wih, np.float32),
                np.asarray(bb, np.float32), fold)
    fc_w = np.asarray(fc_w, np.float32)
    arrs["fc_f"] = np.ascontiguousarray(2.0 * fc_w[:, 0:H].T)       # (20, 4)
    arrs["fc_bw"] = np.ascontiguousarray(2.0 * fc_w[:, H:2 * H].T)  # (20, 4)
    arrs["fc_bias"] = np.asarray(fc_b, np.float32).reshape(1, 4)
    return arrs


def input_specs(t_len):
    specs = {"w_0_0": (97, 116), "w_0_1": (97, 116),
             "x_f": (2, t_len), "x_r": (2, t_len), "ones1": (1, t_len)}
    for l in (1, 2):
        for d in range(2):
            specs[f"w_{l}_{d}"] = (97, 116)
    specs.update({"fc_f": (H, 4), "fc_bw": (H, 4), "fc_bias": (1, 4)})
    return specs


# ---------------------------------------------------------------- device IR
def emit(ctx: ExitStack, tc: tile.TileContext, ins: dict, y_out, t_len: int,
         k_iters: int):
    nc = tc.nc
    T = t_len
    CH = min(512, T)
    nch = T // CH

    wp = ctx.enter_context(tc.tile_pool(name="wp", bufs=1))
    pp = ctx.enter_context(tc.tile_pool(name="pp", bufs=1, space="PSUM"))

    w = {}
    for name, ap in ins.items():
        dt = F32 if name in ("x_f", "x_r") else F32R
        t = wp.tile(list(ap.shape), dt, tag=name, name=f"in_{name}")
        nc.sync.dma_start(t[:], ap[:])
        w[name] = t

    # persistent per-layer rhs tiles: rows 0..19 h~(t-1)@col t, 20..59 X,
    # 20+D ones
    zscratch = wp.tile([97, T + 1], F32, tag="zscratch")
    nc.vector.memset(zscratch[:], 0.0)
    nc.vector.memset(zscratch[96:97, 0:T], 1.0)
    warm = wp.tile([1, 1], F32, tag="warm")
    nc.scalar.activation(warm[:], zscratch[0:1, 0:1], AF.Sigmoid)
    R = {}
    for l in range(3):
        for d in range(2):
            r = wp.tile([97, T + 1], F32R, tag=f"R_{l}_{d}", name=f"R_{l}_{d}")
            R[l, d] = r
            if l == 0:
                nc.vector.tensor_copy(r[:], zscratch[:])
    nc.vector.tensor_copy(R[0, 0][32:34, 0:T], w["x_f"][:])
    nc.vector.tensor_copy(R[0, 1][32:34, 0:T], w["x_r"][:])

    S1 = {d: wp.tile([84, T], F32, tag=f"S1_{d}", name=f"S1_{d}")
          for d in range(2)}
    SG = {d: wp.tile([52, T], F32, tag=f"SG_{d}", name=f"SG_{d}")
          for d in range(2)}
    U = {d: wp.tile([20, T], F32, tag=f"U_{d}", name=f"U_{d}")
         for d in range(2)}
    CT = {d: wp.tile([20, T], F32, tag=f"CT_{d}", name=f"CT_{d}")
          for d in range(2)}
    SC = {d: wp.tile([84, T], F32, tag=f"SC_{d}", name=f"SC_{d}")
          for d in range(2)}
    hb2r = wp.tile([H, T], F32R, tag="hb2r")
    ysb = wp.tile([4, T], F32, tag="ysb")
    ones = w["ones1"]


    for l in range(3):
        kk = 97
        for it in range(k_iters):
            for d in range(2):
                for ch in range(nch):
                    c0 = ch * CH
                    ps = pp.tile([116, CH], F32, tag=f"ps_{d}_{ch}",
                                 name=f"ps_{d}_{ch}")
                    nc.tensor.matmul(
                        ps[:], w[f"w_{l}_{d}"][:],
                        R[l, d][0:kk, c0:c0 + CH],
                        start=True, stop=True)
                    # f,i,o -> sigmoid;  g -> tanh (same ACT table set)
                    nc.scalar.activation(S1[d][0:84, c0:c0 + CH],
                                         ps[0:84, :], AF.Sigmoid)
                    nc.scalar.activation(SG[d][32:52, c0:c0 + CH],
                                         ps[96:116, :], AF.Tanh)
                    # U = tanh(g) * sig(i)  (plain multiply -> GPSIMD)
                    nc.gpsimd.tensor_tensor(
                        U[d][0:H, c0:c0 + CH], SG[d][32:52, c0:c0 + CH],
                        S1[d][32:52, c0:c0 + CH], ALU.mult)
            for ch in range(nch):
                c0 = ch * CH
                for d in range(2):
                    init = 0.0 if ch == 0 else CT[d][0:H, c0 - 1:c0]
                    nc.vector.tensor_tensor_scan(
                        CT[d][0:H, c0:c0 + CH], S1[d][0:H, c0:c0 + CH],
                        U[d][0:H, c0:c0 + CH], init, ALU.mult, ALU.add)
            for d in range(2):
                # sig(2c) = (tanh(c)+1)/2 ;  h~ = (sig(2c)-.5)*o = h/2
                nc.scalar.activation(SC[d][64:84, 0:T], CT[d][0:H, 0:T],
                                     AF.Sigmoid, scale=2.0)
            for d in range(2):
                for ch in range(nch):
                    c0 = ch * CH
                    nc.vector.scalar_tensor_tensor(
                        R[l, d][0:H, 1 + c0:1 + c0 + CH],
                        SC[d][64:84, c0:c0 + CH], -0.5,
                        S1[d][64:84, c0:c0 + CH], ALU.add, ALU.mult)

        if l < 2:
            nc.vector.tensor_copy(R[l + 1, 0][:], zscratch[:])
            nc.vector.tensor_copy(R[l + 1, 1][:], zscratch[:])
            # layer input at time t is [h_f(t), h_b(t)]; b-tiles store
            # scan order (time T-1-s at col s+1), so time t sits at col T-t
            nc.vector.tensor_copy(R[l + 1, 0][32:52, 0:T],
                                  R[l, 0][0:H, 1:T + 1])
            nc.vector.tensor_copy(R[l + 1, 0][64:84, 0:T],
                                  R[l, 1][0:H, T:0:-1])
            nc.vector.tensor_copy(R[l + 1, 1][32:52, 0:T],
                                  R[l, 0][0:H, T:0:-1])
            nc.vector.tensor_copy(R[l + 1, 1][64:84, 0:T],
                                  R[l, 1][0:H, 1:T + 1])

    # ---- final FC: y = 2*fc_w @ [h~_f; h~_b] + fc_b -> (4, T)
    nc.vector.tensor_copy(hb2r[:, 0:T], R[2, 1][0:H, T:0:-1])
    for ch in range(nch):
        c0 = ch * CH
        ps = pp.tile([4, CH], F32, tag="fcps", name="fcps")
        nc.tensor.matmul(ps[:], w["fc_f"][:],
                         R[2, 0][0:H, c0 + 1:c0 + CH + 1],
                         start=True, stop=False)
        nc.tensor.matmul(ps[:], w["fc_bw"][:],
                         hb2r[:, c0:c0 + CH],
                         start=False, stop=False)
        nc.tensor.matmul(ps[:], w["fc_bias"][:],
                         ones[:, c0:c0 + CH],
                         start=False, stop=True)
        nc.scalar.copy(ysb[:, c0:c0 + CH], ps[:])
    nc.sync.dma_start(y_out[:], ysb[:])


def _split_sem_waits(nc, cap=1):
    """The image's walrus supports at most `cap` sem waits per instruction
    ("Too many sync wait commands"); move extras onto preceding same-engine
    NoOps (engines are in-order, so an earlier wait is strictly stronger)."""
    for f in nc.m.functions:
        for bb in f.blocks:
            newlist = []
            changed = False
            for insn in bb.instructions:
                si = insn.sync_info
                if (si is not None and si.on_wait is not None
                        and len(si.on_wait) > cap
                        and not isinstance(insn, mybir.InstAllEngineBarrier)):
                    waits = list(si.on_wait)
                    extras, keep = waits[:-cap], waits[-cap:]
                    for j in range(0, len(extras), cap):
                        newlist.append(mybir.InstNoOp(
                            name=f"{insn.name}_xw{j}", engine=insn.engine,
                            ins=[], outs=[],
                            sync_info=mybir.SyncInfo(on_wait=extras[j:j + cap],
                                                     on_update=[])))
                    si.on_wait = keep
                    changed = True
                newlist.append(insn)
            if changed:
                bb.instructions = newlist


def build(t_len, k_iters=K_ITERS):
    nc = bass.Bass()
    aps = {}
    for name, shape in input_specs(t_len).items():
        dt = F32 if name in ("x_f", "x_r") else F32R
        aps[name] = nc.declare_dram_parameter(name, list(shape), dt,
                                              isOutput=False)
    y = nc.declare_dram_parameter("y_out", [4, t_len], F32, isOutput=True)
    with tile.TileContext(nc) as tc:
        with ExitStack() as ctx:
            emit(ctx, tc, aps, y, t_len, k_iters)
    _split_sem_waits(nc)
    return nc


# ---------------------------------------------------------------- entrypoint
def run(inputs: dict, t_len=1024, trace=False, k_iters=K_ITERS, **kw):
    arrs = prep_inputs(**inputs, t_len=t_len)
    nc = build(t_len, k_iters)
    in_maps = [arrs] * NCORES
    res = run_bass_kernel_spmd(nc, in_maps, list(range(NCORES)), trace=trace,
                               **kw)
    y = np.asarray(res.results[0]["y_out"])  # (4, t_len)
    return y.T.copy(), res


def kernel(**inputs) -> np.ndarray:
    y, _ = run(inputs, t_len=1024)
    return y.astype(np.float32)


if __name__ == "__main__":
    np.random.seed(1)
    T = int(os.environ.get("BASS_LSTM_T", "1024"))
    print(build(T))


# revision 18
# speedup vs baseline: 1.1715x; 1.1715x over previous
"""Trainium2 Bass kernel for nn_BiLSTM_3410204033194.

The reference computes a 3-layer bidirectional LSTM over (T=1024, B=512,
IN=2) and applies the final FC to out[:, -1, :] — the LAST BATCH ELEMENT
only.  LSTM batch elements are independent, so the full output (T, 4)
depends only on batch index 511: we run the whole 3-layer bidirectional
recurrence for that single sequence on one core (all 8 cores run the same
SPMD program; core 0's output is used).

Instead of a step-by-step scan (latency-bound: ~1.5-2.5us per step x 3072
steps), each layer-direction is solved by PARALLEL-IN-TIME fixed-point
(Picard) iteration, which converges geometrically at the LSTM's
contraction rate (~0.28/sweep on this data; K sweeps give ~0.3^K error,
validated end-to-end in fp64/fp32 prototypes at <3e-6 for K=10):

    H^0 = 0
    repeat K times:
        A   = [H^{k-1} shifted by one step; X; 1] @ Waug     (PE, fp32r)
        S   = sigmoid(A)          all 4 gates; tanh(y)=2*sig(2y)-1 with
                                  the x2 folded into Waug's g columns
        U   = (S_g - .5) * S_i                                (DVE STT)
        C   = scan: c_t = S_f[t]*c_{t-1} + U[t]   (DVE tensor_tensor_scan,
                                                   ONE instr for all T)
        S_c = sigmoid(4*C)                                    (ACT)
        H^k = (S_c - .5) * S_o                                (DVE STT)

All state tensors carry h~ = h/2 and c~ = c/2 (the tanh-as-sigmoid
halves); the x2 is folded into every consumer's weights (W_hh, the next
layer's W_ih, and the FC).

The one-step shift is free: H rows of the GEMM's rhs tile R hold h~(t-1)
at column t (the H-update writes columns 1..T), while the X rows hold
x(t) at column t.  The bias rides in Waug against an all-ones row of R.

Partition layout (hardware rule: operand partition starts in {0,32,64,96},
tensor ops' inputs share a start partition):
  psum quads   dir f: f@0 i@32 o@64 g@96      dir b: i@0 f@32 g@64 o@96
  sig1: ps[0:52]->SA_d[64:116]   f:(f@64,i@96)  b:(i@64,f@96)
  sig2: ps[64:116]->SB_d[64:116] f:(o@64,g@96)  b:(g@64,o@96)
  U_f=(SB_f[96:]-.5)*SA_f[96:]->U[64:84]      U_b=(SB_b[64:84]-.5)*SA_b[64:84]->U[96:116]
  TTS_f: (SA_f[64:84], U[64:84])->CT[0:20]    TTS_b: (SA_b[96:116], U[96:116])->CT[32:52]
  sigc: CT[0:52] -> SC[64:116]
  H_f=(SC[64:84]-.5)*SB_f[64:84]->R_f[0:20,1:] H_b=(SC[96:116]-.5)*SB_b[96:116]->R_b[0:20,1:]
"""
import os
import sys

sys.path.insert(0, "/opt/trn_rl_repo")

import numpy as np
from contextlib import ExitStack

import concourse.bass as bass
import concourse.tile as tile
from concourse import mybir
from concourse.bass_utils import run_bass_kernel_spmd

F32 = mybir.dt.float32
F32R = mybir.dt.float32r
AF = mybir.ActivationFunctionType
ALU = mybir.AluOpType

H = 20
NCORES = 8
K_ITERS = 5
# quad (x32) of each pytorch gate, same for both directions
QUAD = {"f": 0, "i": 1, "o": 2, "g": 3}
GATE_ROWS = {"i": 0, "f": 1, "g": 2, "o": 3}  # row blocks in pytorch weights


# ---------------------------------------------------------------- host prep
def _make_lhsT(w_hh, w_ih, b, h_fold_x):
    """Build the augmented stationary (20+D+1, 116).

    rows 0..19   : W_hh^T * 2           (consumes h~ = h/2)
    rows 20..+D  : W_ih^T * h_fold_x    (2 if the layer input is h~ tiles)
    row  20+D    : bias
    columns      : gate quads per `quad`, g columns additionally * 2
                   (tanh(y) = 2*sigmoid(2y) - 1).
    """
    d = w_ih.shape[1]
    out = np.zeros((97, 116), np.float32)
    for gate, gi in GATE_ROWS.items():
        rows = slice(H * gi, H * (gi + 1))
        c0 = 32 * QUAD[gate]
        out[0:H, c0:c0 + H] = w_hh[rows].T * 2.0
        if d == 2:
            out[32:34, c0:c0 + H] = w_ih[rows].T * h_fold_x
        else:
            out[32:52, c0:c0 + H] = w_ih[rows, 0:H].T * h_fold_x
            out[64:84, c0:c0 + H] = w_ih[rows, H:2 * H].T * h_fold_x
        out[96, c0:c0 + H] = b[rows]
    return out


def prep_inputs(x, w_ih0, w_hh0, b0, w_ih12, w_hh12, b12, fc_w, fc_b, t_len):
    arrs = {}
    xs = np.asarray(x[:t_len, -1, :], np.float32)     # (T, 2)
    arrs["x_f"] = np.ascontiguousarray(xs.T)          # (2, T)
    arrs["x_r"] = np.ascontiguousarray(xs[::-1].T)    # (2, T) reversed time
    arrs["ones1"] = np.ones((1, t_len), np.float32)
    for l in range(3):
        for d in range(2):
            if l == 0:
                wih, whh, bb = w_ih0[d], w_hh0[d], b0[d]
                fold = 1.0
            else:
                wih, whh, bb = w_ih12[l - 1, d], w_hh12[l - 1, d], b12[l - 1, d]
                fold = 2.0
            arrs[f"w_{l}_{d}"] = _make_lhsT(
                np.asarray(whh, np.float32), np.asarray(wih, np.float32),
                np.asarray(bb, np.float32), fold)
    fc_w = np.asarray(fc_w, np.float32)
    arrs["fc_f"] = np.ascontiguousarray(2.0 * fc_w[:, 0:H].T)       # (20, 4)
    arrs["fc_bw"] = np.ascontiguousarray(2.0 * fc_w[:, H:2 * H].T)  # (20, 4)
    arrs["fc_bias"] = np.asarray(fc_b, np.float32).reshape(1, 4)
    return arrs


def input_specs(t_len):
    specs = {"w_0_0": (97, 116), "w_0_1": (97, 116),
             "x_f": (2, t_len), "x_r": (2, t_len), "ones1": (1, t_len)}
    for l in (1, 2):
        for d in range(2):
            specs[f"w_{l}_{d}"] = (97, 116)
    specs.update({"fc_f": (H, 4), "fc_bw": (H, 4), "fc_bias": (1, 4)})
    return specs


# ---------------------------------------------------------------- device IR
def emit(ctx: ExitStack, tc: tile.TileContext, ins: dict, y_out, t_len: int,
         k_iters: int):
    nc = tc.nc
    T = t_len
    CH = min(512, T)
    nch = T // CH

    wp = ctx.enter_context(tc.tile_pool(name="wp", bufs=1))
    pp = ctx.enter_context(tc.tile_pool(name="pp", bufs=1, space="PSUM"))

    w = {}
    for name, ap in ins.items():
        dt = F32 if name in ("x_f", "x_r") else F32R
        t = wp.tile(list(ap.shape), dt, tag=name, name=f"in_{name}")
        nc.sync.dma_start(t[:], ap[:])
        w[name] = t

    # persistent per-layer rhs tiles: rows 0..19 h~(t-1)@col t, 20..59 X,
    # 20+D ones
    zscratch = wp.tile([97, T + 1], F32, tag="zscratch")
    nc.gpsimd.memset(zscratch[:], 0.0)
    nc.gpsimd.memset(zscratch[96:97, 0:T], 1.0)
    warm = wp.tile([1, 1], F32, tag="warm")
    nc.scalar.activation(warm[:], zscratch[0:1, 0:1], AF.Sigmoid)
    R = {}
    for l in range(3):
        for d in range(2):
            r = wp.tile([97, T + 1], F32R, tag=f"R_{l}_{d}", name=f"R_{l}_{d}")
            R[l, d] = r
            if l == 0:
                nc.gpsimd.tensor_copy(r[:], zscratch[:])
    nc.scalar.copy(R[0, 0][32:34, 0:T], w["x_f"][:])
    nc.scalar.copy(R[0, 1][32:34, 0:T], w["x_r"][:])

    S1 = {d: wp.tile([84, T], F32, tag=f"S1_{d}", name=f"S1_{d}")
          for d in range(2)}
    SG = {d: wp.tile([52, T], F32, tag=f"SG_{d}", name=f"SG_{d}")
          for d in range(2)}
    U = {d: wp.tile([20, T], F32, tag=f"U_{d}", name=f"U_{d}")
         for d in range(2)}
    CT = {d: wp.tile([20, T], F32, tag=f"CT_{d}", name=f"CT_{d}")
          for d in range(2)}
    SC = {d: wp.tile([84, T], F32, tag=f"SC_{d}", name=f"SC_{d}")
          for d in range(2)}
    hb2r = wp.tile([H, T], F32R, tag="hb2r")
    ysb = wp.tile([4, T], F32, tag="ysb")
    ones = w["ones1"]


    for l in range(3):
        kk = 97
        for it in range(k_iters):
            for d in range(2):
                for ch in range(nch):
                    c0 = ch * CH
                    ps = pp.tile([116, CH], F32, tag=f"ps_{d}_{ch}",
                                 name=f"ps_{d}_{ch}")
                    nc.tensor.matmul(
                        ps[:], w[f"w_{l}_{d}"][:],
                        R[l, d][0:kk, c0:c0 + CH],
                        start=True, stop=True)
                    # f,i,o -> sigmoid;  g -> tanh (same ACT table set)
                    nc.scalar.activation(S1[d][0:84, c0:c0 + CH],
                                         ps[0:84, :], AF.Sigmoid)
                    nc.scalar.activation(SG[d][32:52, c0:c0 + CH],
                                         ps[96:116, :], AF.Tanh)
            for d in range(2):
                # U = tanh(g) * sig(i); TTS has a ~2.3us fixed cost so one
                # full-T scan per direction is the cheapest shape
                nc.vector.tensor_tensor(
                    U[d][0:H, 0:T], SG[d][32:52, 0:T],
                    S1[d][32:52, 0:T], ALU.mult)
                nc.vector.tensor_tensor_scan(
                    CT[d][0:H, 0:T], S1[d][0:H, 0:T],
                    U[d][0:H, 0:T], 0.0, ALU.mult, ALU.add)
            for d in range(2):
                # sig(2c) = (tanh(c)+1)/2 ;  h~ = (sig(2c)-.5)*o = h/2
                nc.scalar.activation(SC[d][64:84, 0:T], CT[d][0:H, 0:T],
                                     AF.Sigmoid, scale=2.0)
            for d in range(2):
                for ch in range(nch):
                    c0 = ch * CH
                    nc.vector.scalar_tensor_tensor(
                        R[l, d][0:H, 1 + c0:1 + c0 + CH],
                        SC[d][64:84, c0:c0 + CH], -0.5,
                        S1[d][64:84, c0:c0 + CH], ALU.add, ALU.mult)

        if l < 2:
            nc.gpsimd.tensor_copy(R[l + 1, 0][:], zscratch[:])
            nc.gpsimd.tensor_copy(R[l + 1, 1][:], zscratch[:])
            # layer input at time t is [h_f(t), h_b(t)]; b-tiles store
            # scan order (time T-1-s at col s+1), so time t sits at col T-t
            nc.scalar.copy(R[l + 1, 0][32:52, 0:T],
                           R[l, 0][0:H, 1:T + 1])
            nc.gpsimd.tensor_copy(R[l + 1, 0][64:84, 0:T],
                                  R[l, 1][0:H, T:0:-1])
            nc.gpsimd.tensor_copy(R[l + 1, 1][32:52, 0:T],
                                  R[l, 0][0:H, T:0:-1])
            nc.scalar.copy(R[l + 1, 1][64:84, 0:T],
                           R[l, 1][0:H, 1:T + 1])

    # ---- final FC: y = 2*fc_w @ [h~_f; h~_b] + fc_b -> (4, T)
    nc.scalar.copy(hb2r[:, 0:T], R[2, 1][0:H, T:0:-1])
    for ch in range(nch):
        c0 = ch * CH
        ps = pp.tile([4, CH], F32, tag="fcps", name="fcps")
        nc.tensor.matmul(ps[:], w["fc_f"][:],
                         R[2, 0][0:H, c0 + 1:c0 + CH + 1],
                         start=True, stop=False)
        nc.tensor.matmul(ps[:], w["fc_bw"][:],
                         hb2r[:, c0:c0 + CH],
                         start=False, stop=False)
        nc.tensor.matmul(ps[:], w["fc_bias"][:],
                         ones[:, c0:c0 + CH],
                         start=False, stop=True)
        nc.scalar.copy(ysb[:, c0:c0 + CH], ps[:])
    nc.sync.dma_start(y_out[:], ysb[:])


def _split_sem_waits(nc, cap=1):
    """The image's walrus supports at most `cap` sem waits per instruction
    ("Too many sync wait commands"); move extras onto preceding same-engine
    NoOps (engines are in-order, so an earlier wait is strictly stronger)."""
    for f in nc.m.functions:
        for bb in f.blocks:
            newlist = []
            changed = False
            for insn in bb.instructions:
                si = insn.sync_info
                if (si is not None and si.on_wait is not None
                        and len(si.on_wait) > cap
                        and not isinstance(insn, mybir.InstAllEngineBarrier)):
                    waits = list(si.on_wait)
                    extras, keep = waits[:-cap], waits[-cap:]
                    for j in range(0, len(extras), cap):
                        newlist.append(mybir.InstNoOp(
                            name=f"{insn.name}_xw{j}", engine=insn.engine,
                            ins=[], outs=[],
                            sync_info=mybir.SyncInfo(on_wait=extras[j:j + cap],
                                                     on_update=[])))
                    si.on_wait = keep
                    changed = True
                newlist.append(insn)
            if changed:
                bb.instructions = newlist


def build(t_len, k_iters=K_ITERS):
    nc = bass.Bass()
    aps = {}
    for name, shape in input_specs(t_len).items():
        dt = F32 if name in ("x_f", "x_r") else F32R
        aps[name] = nc.declare_dram_parameter(name, list(shape), dt,
                                              isOutput=False)
    y = nc.declare_dram_parameter("y_out", [4, t_len], F32, isOutput=True)
    with tile.TileContext(nc) as tc:
        with ExitStack() as ctx:
            emit(ctx, tc, aps, y, t_len, k_iters)
    _split_sem_waits(nc)
    return nc


# ---------------------------------------------------------------- entrypoint
def run(inputs: dict, t_len=1024, trace=False, k_iters=K_ITERS, **kw):
    arrs = prep_inputs(**inputs, t_len=t_len)
    nc = build(t_len, k_iters)
    in_maps = [arrs] * NCORES
    res = run_bass_kernel_spmd(nc, in_maps, list(range(NCORES)), trace=trace,
                               **kw)
    y = np.asarray(res.results[0]["y_out"])  # (4, t_len)
    return y.T.copy(), res


def kernel(**inputs) -> np.ndarray:
    y, _ = run(inputs, t_len=1024)
    return y.astype(np.float32)


if __name__ == "__main__":
    np.random.seed(1)
    T = int(os.environ.get("BASS_LSTM_T", "1024"))
    print(build(T))


# revision 23
# speedup vs baseline: 1.2414x; 1.0597x over previous
"""Trainium2 Bass kernel for nn_BiLSTM_3410204033194.

The reference computes a 3-layer bidirectional LSTM over (T=1024, B=512,
IN=2) and applies the final FC to out[:, -1, :] — the LAST BATCH ELEMENT
only.  LSTM batch elements are independent, so the full output (T, 4)
depends only on batch index 511: we run the whole 3-layer bidirectional
recurrence for that single sequence on one core (all 8 cores run the same
SPMD program; core 0's output is used).

Instead of a step-by-step scan (latency-bound: ~1.5-2.5us per step x 3072
steps), each layer-direction is solved by PARALLEL-IN-TIME fixed-point
(Picard) iteration, which converges geometrically at the LSTM's
contraction rate (~0.28/sweep on this data; K sweeps give ~0.3^K error,
validated end-to-end in fp64/fp32 prototypes at <3e-6 for K=10):

    H^0 = 0
    repeat K times:
        A   = [H^{k-1} shifted by one step; X; 1] @ Waug     (PE, fp32r)
        S   = sigmoid(A)          all 4 gates; tanh(y)=2*sig(2y)-1 with
                                  the x2 folded into Waug's g columns
        U   = (S_g - .5) * S_i                                (DVE STT)
        C   = scan: c_t = S_f[t]*c_{t-1} + U[t]   (DVE tensor_tensor_scan,
                                                   ONE instr for all T)
        S_c = sigmoid(4*C)                                    (ACT)
        H^k = (S_c - .5) * S_o                                (DVE STT)

All state tensors carry h~ = h/2 and c~ = c/2 (the tanh-as-sigmoid
halves); the x2 is folded into every consumer's weights (W_hh, the next
layer's W_ih, and the FC).

The one-step shift is free: H rows of the GEMM's rhs tile R hold h~(t-1)
at column t (the H-update writes columns 1..T), while the X rows hold
x(t) at column t.  The bias rides in Waug against an all-ones row of R.

Partition layout (hardware rule: operand partition starts in {0,32,64,96},
tensor ops' inputs share a start partition):
  psum quads   dir f: f@0 i@32 o@64 g@96      dir b: i@0 f@32 g@64 o@96
  sig1: ps[0:52]->SA_d[64:116]   f:(f@64,i@96)  b:(i@64,f@96)
  sig2: ps[64:116]->SB_d[64:116] f:(o@64,g@96)  b:(g@64,o@96)
  U_f=(SB_f[96:]-.5)*SA_f[96:]->U[64:84]      U_b=(SB_b[64:84]-.5)*SA_b[64:84]->U[96:116]
  TTS_f: (SA_f[64:84], U[64:84])->CT[0:20]    TTS_b: (SA_b[96:116], U[96:116])->CT[32:52]
  sigc: CT[0:52] -> SC[64:116]
  H_f=(SC[64:84]-.5)*SB_f[64:84]->R_f[0:20,1:] H_b=(SC[96:116]-.5)*SB_b[96:116]->R_b[0:20,1:]
"""
import os
import sys

sys.path.insert(0, "/opt/trn_rl_repo")

import numpy as np
from contextlib import ExitStack

import concourse.bass as bass
import concourse.tile as tile
from concourse import mybir
from concourse.bass_utils import run_bass_kernel_spmd

F32 = mybir.dt.float32
F32R = mybir.dt.float32r
AF = mybir.ActivationFunctionType
ALU = mybir.AluOpType

H = 20
NCORES = 8
K_ITERS = 5
# quad (x32) of each pytorch gate, same for both directions
QUAD = {"f": 0, "i": 1, "o": 2, "g": 3}
GATE_ROWS = {"i": 0, "f": 1, "g": 2, "o": 3}  # row blocks in pytorch weights


# ---------------------------------------------------------------- host prep
def _make_lhsT(w_hh, w_ih, b, h_fold_x):
    """Build the augmented stationary (20+D+1, 116).

    rows 0..19   : W_hh^T * 2           (consumes h~ = h/2)
    rows 20..+D  : W_ih^T * h_fold_x    (2 if the layer input is h~ tiles)
    row  20+D    : bias
    columns      : gate quads per `quad`, g columns additionally * 2
                   (tanh(y) = 2*sigmoid(2y) - 1).
    """
    d = w_ih.shape[1]
    out = np.zeros((97, 116), np.float32)
    for gate, gi in GATE_ROWS.items():
        rows = slice(H * gi, H * (gi + 1))
        c0 = 32 * QUAD[gate]
        out[0:H, c0:c0 + H] = w_hh[rows].T * 2.0
        if d == 2:
            out[32:34, c0:c0 + H] = w_ih[rows].T * h_fold_x
        else:
            out[32:52, c0:c0 + H] = w_ih[rows, 0:H].T * h_fold_x
            out[64:84, c0:c0 + H] = w_ih[rows, H:2 * H].T * h_fold_x
        out[96, c0:c0 + H] = b[rows]
    return out


def prep_inputs(x, w_ih0, w_hh0, b0, w_ih12, w_hh12, b12, fc_w, fc_b, t_len):
    arrs = {}
    xs = np.asarray(x[:t_len, -1, :], np.float32)     # (T, 2)
    xcat = np.zeros((34, t_len), np.float32)
    xcat[0:2] = xs.T
    xcat[32:34] = xs[::-1].T
    arrs["xcat"] = xcat
    for l in range(3):
        for d in range(2):
            if l == 0:
                wih, whh, bb = w_ih0[d], w_hh0[d], b0[d]
                fold = 1.0
            else:
                wih, whh, bb = w_ih12[l - 1, d], w_hh12[l - 1, d], b12[l - 1, d]
                fold = 2.0
            arrs[f"w_{l}_{d}"] = _make_lhsT(
                np.asarray(whh, np.float32), np.asarray(wih, np.float32),
                np.asarray(bb, np.float32), fold)
    wcat = np.concatenate(
        [arrs.pop(f"w_{l}_{d}") for l in range(3) for d in range(2)], axis=1)
    arrs["wcat"] = np.ascontiguousarray(wcat)         # (97, 696)
    fc_w = np.asarray(fc_w, np.float32)
    arrs["fc_f"] = np.ascontiguousarray(2.0 * fc_w[:, 0:H].T)
    arrs["fc_bw"] = np.ascontiguousarray(2.0 * fc_w[:, H:2 * H].T)
    arrs["fc_bias"] = np.asarray(fc_b, np.float32).reshape(1, 4)
    arrs["ones1"] = np.ones((1, t_len), np.float32)
    return arrs


def input_specs(t_len):
    return {"wcat": (97, 696), "xcat": (34, t_len), "fc_f": (H, 4),
            "fc_bw": (H, 4), "fc_bias": (1, 4), "ones1": (1, t_len)}


# ---------------------------------------------------------------- device IR
def emit(ctx: ExitStack, tc: tile.TileContext, ins: dict, y_out, t_len: int,
         k_iters: int):
    nc = tc.nc
    T = t_len
    CH = min(512, T)
    nch = T // CH

    wp = ctx.enter_context(tc.tile_pool(name="wp", bufs=1))
    pp = ctx.enter_context(tc.tile_pool(name="pp", bufs=1, space="PSUM"))

    w = {}
    for name, ap in ins.items():
        dt = F32 if name == "xcat" else F32R
        t = wp.tile(list(ap.shape), dt, tag=name, name=f"in_{name}")
        nc.sync.dma_start(t[:], ap[:])
        w[name] = t

    def lhsT(l, d):
        c = (2 * l + d) * 116
        return w["wcat"][0:97, c:c + 116]

    # persistent per-layer rhs tiles: rows 0..19 h~(t-1)@col t, 20..59 X,
    # 20+D ones
    zscratch = wp.tile([97, T + 1], F32, tag="zscratch")
    nc.vector.memset(zscratch[:], 0.0)
    nc.vector.memset(zscratch[96:97, 0:T], 1.0)
    warm = wp.tile([1, 1], F32, tag="warm")
    nc.scalar.activation(warm[:], zscratch[0:1, 0:1], AF.Sigmoid)
    R = {}
    for l in range(3):
        for d in range(2):
            r = wp.tile([97, T + 1], F32R, tag=f"R_{l}_{d}", name=f"R_{l}_{d}")
            R[l, d] = r
            if l == 0:
                nc.vector.tensor_copy(r[:], zscratch[:])
    nc.vector.tensor_copy(R[0, 0][32:34, 0:T], w["xcat"][0:2, :])
    nc.vector.tensor_copy(R[0, 1][32:34, 0:T], w["xcat"][32:34, :])

    S1 = {d: wp.tile([84, T], F32, tag=f"S1_{d}", name=f"S1_{d}")
          for d in range(2)}
    SG = {d: wp.tile([52, T], F32, tag=f"SG_{d}", name=f"SG_{d}")
          for d in range(2)}
    U = {d: wp.tile([20, T], F32, tag=f"U_{d}", name=f"U_{d}")
         for d in range(2)}
    CT = {d: wp.tile([20, T], F32, tag=f"CT_{d}", name=f"CT_{d}")
          for d in range(2)}
    SC = {d: wp.tile([84, T], F32, tag=f"SC_{d}", name=f"SC_{d}")
          for d in range(2)}
    hb2r = wp.tile([H, T], F32R, tag="hb2r")
    ysb = wp.tile([4, T], F32, tag="ysb")


    for l in range(3):
        kk = 97
        for it in range(k_iters):
            for d in range(2):
                for ch in range(nch):
                    c0 = ch * CH
                    ps = pp.tile([116, CH], F32, tag=f"ps_{d}_{ch}",
                                 name=f"ps_{d}_{ch}")
                    nc.tensor.matmul(
                        ps[:], lhsT(l, d),
                        R[l, d][0:kk, c0:c0 + CH],
                        start=True, stop=True)
                    # f,i,o -> sigmoid;  g -> tanh (same ACT table set)
                    nc.scalar.activation(S1[d][0:84, c0:c0 + CH],
                                         ps[0:84, :], AF.Sigmoid)
                    nc.scalar.activation(SG[d][32:52, c0:c0 + CH],
                                         ps[96:116, :], AF.Tanh)
            for d in range(2):
                # U = tanh(g) * sig(i); TTS has a ~2.3us fixed cost so one
                # full-T scan per direction is the cheapest shape
                nc.vector.tensor_tensor(
                    U[d][0:H, 0:T], SG[d][32:52, 0:T],
                    S1[d][32:52, 0:T], ALU.mult)
                nc.vector.tensor_tensor_scan(
                    CT[d][0:H, 0:T], S1[d][0:H, 0:T],
                    U[d][0:H, 0:T], 0.0, ALU.mult, ALU.add)
            for d in range(2):
                # sig(2c) = (tanh(c)+1)/2 ;  h~ = (sig(2c)-.5)*o = h/2
                nc.scalar.activation(SC[d][64:84, 0:T], CT[d][0:H, 0:T],
                                     AF.Sigmoid, scale=2.0)
            for d in range(2):
                for ch in range(nch):
                    c0 = ch * CH
                    nc.vector.scalar_tensor_tensor(
                        R[l, d][0:H, 1 + c0:1 + c0 + CH],
                        SC[d][64:84, c0:c0 + CH], -0.5,
                        S1[d][64:84, c0:c0 + CH], ALU.add, ALU.mult)

        if l < 2:
            nc.vector.tensor_copy(R[l + 1, 0][:], zscratch[:])
            nc.vector.tensor_copy(R[l + 1, 1][:], zscratch[:])
            # layer input at time t is [h_f(t), h_b(t)]; b-tiles store
            # scan order (time T-1-s at col s+1), so time t sits at col T-t
            nc.vector.tensor_copy(R[l + 1, 0][32:52, 0:T],
                                  R[l, 0][0:H, 1:T + 1])
            nc.vector.tensor_copy(R[l + 1, 0][64:84, 0:T],
                                  R[l, 1][0:H, T:0:-1])
            nc.vector.tensor_copy(R[l + 1, 1][32:52, 0:T],
                                  R[l, 0][0:H, T:0:-1])
            nc.vector.tensor_copy(R[l + 1, 1][64:84, 0:T],
                                  R[l, 1][0:H, 1:T + 1])

    # ---- final FC: y = 2*fc_w @ [h~_f; h~_b] + fc_b -> (4, T)
    nc.vector.tensor_copy(hb2r[:, 0:T], R[2, 1][0:H, T:0:-1])
    for ch in range(nch):
        c0 = ch * CH
        ps = pp.tile([4, CH], F32, tag="fcps", name="fcps")
        nc.tensor.matmul(ps[:], w["fc_f"][:],
                         R[2, 0][0:H, c0 + 1:c0 + CH + 1],
                         start=True, stop=False)
        nc.tensor.matmul(ps[:], w["fc_bw"][:],
                         hb2r[:, c0:c0 + CH],
                         start=False, stop=False)
        nc.tensor.matmul(ps[:], w["fc_bias"][:],
                         w["ones1"][:, c0:c0 + CH],
                         start=False, stop=True)
        nc.scalar.copy(ysb[:, c0:c0 + CH], ps[:])
    nc.sync.dma_start(y_out[:], ysb[:])


def _split_sem_waits(nc, cap=1):
    """The image's walrus supports at most `cap` sem waits per instruction
    ("Too many sync wait commands"); move extras onto preceding same-engine
    NoOps (engines are in-order, so an earlier wait is strictly stronger)."""
    for f in nc.m.functions:
        for bb in f.blocks:
            newlist = []
            changed = False
            for insn in bb.instructions:
                si = insn.sync_info
                if (si is not None and si.on_wait is not None
                        and len(si.on_wait) > cap
                        and not isinstance(insn, mybir.InstAllEngineBarrier)):
                    waits = list(si.on_wait)
                    extras, keep = waits[:-cap], waits[-cap:]
                    for j in range(0, len(extras), cap):
                        newlist.append(mybir.InstNoOp(
                            name=f"{insn.name}_xw{j}", engine=insn.engine,
                            ins=[], outs=[],
                            sync_info=mybir.SyncInfo(on_wait=extras[j:j + cap],
                                                     on_update=[])))
                    si.on_wait = keep
                    changed = True
                newlist.append(insn)
            if changed:
                bb.instructions = newlist


def build(t_len, k_iters=K_ITERS):
    nc = bass.Bass()
    aps = {}
    for name, shape in input_specs(t_len).items():
        dt = F32 if name == "xcat" else F32R
        aps[name] = nc.declare_dram_parameter(name, list(shape), dt,
                                              isOutput=False)
    y = nc.declare_dram_parameter("y_out", [4, t_len], F32, isOutput=True)
    with tile.TileContext(nc) as tc:
        with ExitStack() as ctx:
            emit(ctx, tc, aps, y, t_len, k_iters)
    _split_sem_waits(nc)
    return nc


# ---------------------------------------------------------------- entrypoint
def run(inputs: dict, t_len=1024, trace=False, k_iters=K_ITERS, **kw):
    arrs = prep_inputs(**inputs, t_len=t_len)
    nc = build(t_len, k_iters)
    in_maps = [arrs] * NCORES
    res = run_bass_kernel_spmd(nc, in_maps, list(range(NCORES)), trace=trace,
                               **kw)
    y = np.asarray(res.results[0]["y_out"])  # (4, t_len)
    return y.T.copy(), res


def kernel(**inputs) -> np.ndarray:
    y, _ = run(inputs, t_len=1024)
    return y.astype(np.float32)


if __name__ == "__main__":
    np.random.seed(1)
    T = int(os.environ.get("BASS_LSTM_T", "1024"))
    print(build(T))
